# revision 24
# baseline (speedup 1.0000x reference)
"""Distributed GRACE-style contrastive loss on 8 Trainium2 NeuronCores.

Math (reference):
    h = elu(z @ W1 + b1) @ W2 + b2           for z1, z2    -> h1, h2
    hn = h / max(||h||_row, eps)
    S11 = h1n @ h1n.T, S22 = h2n @ h2n.T, S12 = h1n @ h2n.T   (N x N)
    denom1_i = sum_j e^{2 S11_ij} + sum_j e^{2 S12_ij} - e^{2 S11_ii}
    denom2_i = sum_j e^{2 S22_ij} + sum_j e^{2 S12_ji} - e^{2 S22_ii}
    loss = mean_i [ 0.5 (log denom1_i + log denom2_i) - 2 S12_ii ]

Strategy: shard rows across 8 cores.  Each core projects+normalizes its
1024 rows (transposed layout, features on partitions; fp8 DoubleRow
matmuls with x16-scaled weights, biases and the 1/16 descale folded
into the inter-layer vector ops), emits the normalized block both as
bf16 (for the exact S12 diagonal) and as x16-scaled fp8e4 in DoubleRow
pair layout, AllGathers the fp8 blocks (one collective per tensor so
the first overlaps the second projection), then computes its row-block
of the three similarity matrices with fp8 DoubleRow matmuls (K=256 per
instruction) and fused exp+row-reduction on the scalar engine.  The
x256 dot-product scale is folded into the exp.  The S11 pass only needs
the first AllGather, so it runs while the second is in flight.  Column
sums of exp(2 S12) accumulate on the vector engine and are
partition-reduced at the end; the final log/mean combine runs on the
host from tiny per-core outputs.
"""

import sys

sys.path.insert(0, "/opt/trn_rl_repo")

import numpy as np
from concourse import bacc, mybir, tile
from concourse.bass_utils import run_bass_kernel_spmd

F32 = mybir.dt.float32
BF16 = mybir.dt.bfloat16
FP8 = mybir.dt.float8e4
AF = mybir.ActivationFunctionType
ALU = mybir.AluOpType
DR = mybir.MatmulPerfMode.DoubleRow

N = 8192          # total rows
D = 512           # hidden dim (= proj dim)
NCORES = 8
NL = N // NCORES  # 1024 local rows per core
TAU = 0.5
SIGMA = 16.0      # fp8 pre-scale; S accumulates SIGMA^2 * S_true
SCALE_DEV = (1.0 / TAU) / (SIGMA * SIGMA)  # exp scale on device
NDC = D // 128    # 4 feature chunks of 128 partitions
NQ = 2            # two K=256 DoubleRow groups
NIT = NL // 128   # 8 local row tiles of 128
EPS = 1e-12

_CACHE = {}


def _build():
    nc = bacc.Bacc("TRN2", target_bir_lowering=False, debug=False,
                   num_devices=NCORES)

    # ---- I/O ----------------------------------------------------------
    # z and the x16-scaled weights come pre-packed in fp8 DoubleRow pair
    # layout [q, p, pair, cols] with contraction index d = q*256 + pair*128 + p
    z1t_d = nc.declare_dram_parameter("z1f8", [NQ, 128, NQ, NL], FP8, isOutput=False)
    z2t_d = nc.declare_dram_parameter("z2f8", [NQ, 128, NQ, NL], FP8, isOutput=False)
    w1_d = nc.declare_dram_parameter("w1f8", [NQ, 128, NQ, D], FP8, isOutput=False)
    # biases pre-multiplied by 16 on the host; (psum + 16 b)/16 descales the
    # x16 fp8 weight scale in a single two-scalar vector op
    b1_d = nc.declare_dram_parameter("b1s", [D, 1], F32, isOutput=False)
    w2_d = nc.declare_dram_parameter("w2f8", [NQ, 128, NQ, D], FP8, isOutput=False)
    # b2 adjusted on host: b2 - colsum(W2) (folds the elu()+1 shift back out)
    b2_d = nc.declare_dram_parameter("b2s", [D, 1], F32, isOutput=False)

    # rowsum(exp 2*S11)+rowsum(exp 2*S12) and rowsum(exp 2*S22); [p, it]
    out_rs1 = nc.declare_dram_parameter("out_rs1", [128, NIT], F32, isOutput=True)
    out_rs22 = nc.declare_dram_parameter("out_rs22", [128, NIT], F32, isOutput=True)
    out_diag = nc.declare_dram_parameter("out_diag", [1, NL], F32, isOutput=True)
    out_cs = nc.declare_dram_parameter("out_cs", [1, N], F32, isOutput=True)

    with tile.TileContext(nc) as tc:
        with (
            tc.tile_pool(name="const", bufs=1) as constp,
            tc.tile_pool(name="locals", bufs=1) as localp,
            tc.tile_pool(name="accs", bufs=1) as accp,
            tc.tile_pool(name="escratch", bufs=8) as ep,
            tc.tile_pool(name="psbig", bufs=3, space="PSUM") as pp,
            tc.tile_pool(name="pssmall", bufs=1, space="PSUM") as psn_pool,
            tc.tile_pool(name="dram", bufs=1, space="DRAM") as dramp,
        ):
            ones_col = constp.tile([128, 1], F32)
            nc.vector.memset(ones_col[:], 1.0)
            ones_row = constp.tile([1, 128], F32)
            nc.vector.memset(ones_row[:], 1.0)
            ones_col_bf = constp.tile([128, 1], BF16)
            nc.vector.memset(ones_col_bf[:], 1.0)

            w1_sb = []
            w2_sb = []
            for q in range(NQ):
                w1t = constp.tile([128, NQ, D], FP8, name=f"w1_{q}")
                nc.sync.dma_start(w1t[:], w1_d[q])
                w1_sb.append(w1t)
                w2t = constp.tile([128, NQ, D], FP8, name=f"w2_{q}")
                nc.sync.dma_start(w2t[:], w2_d[q])
                w2_sb.append(w2t)
            b1_sb = constp.tile([128, NDC], F32)
            b2_sb = constp.tile([128, NDC], F32)
            for dc in range(NDC):
                nc.gpsimd.dma_start(b1_sb[:, dc:dc + 1], b1_d[dc * 128:(dc + 1) * 128, :])
                nc.gpsimd.dma_start(b2_sb[:, dc:dc + 1], b2_d[dc * 128:(dc + 1) * 128, :])

            # normalized local embeddings, transposed: bf16 [d, i] for the
            # diagonal, and x16-scaled fp8 in DoubleRow pair layout
            # [p, pair, i] with d = q*256 + pair*128 + p
            lns = [[localp.tile([128, NL], BF16, name=f"ln{t}_{dc}")
                    for dc in range(NDC)] for t in range(2)]
            lf8 = [[localp.tile([128, NQ, NL], FP8, name=f"lf8_{t}_{q}")
                    for q in range(NQ)] for t in range(2)]

            cc_in = [dramp.tile([NQ, 128, NQ, NL], FP8, name=f"cc_in{t}")
                     for t in range(2)]
            cc_out = [dramp.tile([NCORES, NQ, 128, NQ, NL], FP8,
                                 addr_space="Shared", name=f"cc_out{t}")
                      for t in range(2)]

            # ---- Phase A: projection + normalize ----------------------
            # ACT kept to Exp/Sqrt only (table swaps are expensive);
            # relu/square run on the vector engine.
            with (
                tc.tile_pool(name="zpool", bufs=2) as zp,
                tc.tile_pool(name="elupool", bufs=2) as elup,
                tc.tile_pool(name="hpool", bufs=2) as hp,
                tc.tile_pool(name="rnpool", bufs=2) as rnp,
            ):
                for t, zt_d in ((0, z1t_d), (1, z2t_d)):
                    zt = []
                    for q in range(NQ):
                        z = zp.tile([128, NQ, NL], FP8, tag=f"z{q}", name=f"z{t}_{q}")
                        nc.sync.dma_start(z[:], zt_d[q])
                        zt.append(z)
                    # layer 1: a.T[p, i] = 16 W1[d,p].T @ z.T[d, i], fp8 DR
                    elus = [elup.tile([128, NQ, NL], FP8, tag=f"el{q}",
                                      name=f"el{t}{q}") for q in range(NQ)]
                    for pc in range(NDC):
                        ps_a = pp.tile([128, 2, 512], F32, tag="ps",
                                       name=f"psa{t}{pc}")
                        for ihh in range(2):
                            for q in range(NQ):
                                nc.tensor.matmul(
                                    ps_a[:, ihh, :],
                                    w1_sb[q][:, :, pc * 128:(pc + 1) * 128],
                                    zt[q][:, :, ihh * 512:(ihh + 1) * 512],
                                    start=q == 0, stop=q == NQ - 1,
                                    perf_mode=DR)
                        # elu(x)+1 = relu(x) + exp(min(x, 0)),  x = (a+16b1)/16
                        xp_sb = ep.tile([128, 2, 512], F32, tag="e",
                                        name=f"xp{t}{pc}")
                        nc.vector.tensor_scalar(
                            xp_sb[:], ps_a[:], b1_sb[:, pc:pc + 1], 1.0 / SIGMA,
                            op0=ALU.add, op1=ALU.mult)
                        r_sb = ep.tile([128, 2, 512], F32, tag="e",
                                       name=f"r{t}{pc}")
                        nc.vector.tensor_scalar(
                            r_sb[:], xp_sb[:], 0.0, None, op0=ALU.max)
                        m_sb = ep.tile([128, 2, 512], F32, tag="e",
                                       name=f"m{t}{pc}")
                        nc.vector.tensor_scalar(
                            m_sb[:], xp_sb[:], 0.0, None, op0=ALU.min)
                        x_sb = ep.tile([128, 2, 512], F32, tag="e",
                                       name=f"x{t}{pc}")
                        nc.scalar.activation(x_sb[:], m_sb[:], AF.Exp)
                        q, pair = divmod(pc, 2)
                        for ihh in range(2):
                            nc.vector.tensor_tensor(
                                elus[q][:, pair, ihh * 512:(ihh + 1) * 512],
                                r_sb[:, ihh, :], x_sb[:, ihh, :], op=ALU.add)
                    # layer 2: h.T[o, i] = 16 W2[p,o].T @ elup1.T[p, i], fp8 DR
                    ps_n = psn_pool.tile([1, 2, 512], F32, tag="pssm",
                                         name=f"psn{t}")
                    hts = []
                    for oc in range(NDC):
                        ps_h = pp.tile([128, 2, 512], F32, tag="ps",
                                       name=f"psh{t}{oc}")
                        for ihh in range(2):
                            for q in range(NQ):
                                nc.tensor.matmul(
                                    ps_h[:, ihh, :],
                                    w2_sb[q][:, :, oc * 128:(oc + 1) * 128],
                                    elus[q][:, :, ihh * 512:(ihh + 1) * 512],
                                    start=q == 0, stop=q == NQ - 1,
                                    perf_mode=DR)
                        ht = hp.tile([128, 2, 512], F32, tag=f"h{oc}",
                                     name=f"h{t}{oc}")
                        nc.vector.tensor_scalar(
                            ht[:], ps_h[:], b2_sb[:, oc:oc + 1], 1.0 / SIGMA,
                            op0=ALU.add, op1=ALU.mult)
                        sq = ep.tile([128, 2, 512], F32, tag="e",
                                     name=f"sq{t}{oc}")
                        nc.vector.tensor_tensor(sq[:], ht[:], ht[:],
                                                op=ALU.mult)
                        hts.append(ht)
                        for ihh in range(2):
                            nc.tensor.matmul(ps_n[:, ihh, :], ones_col[:],
                                             sq[:, ihh, :],
                                             start=oc == 0, stop=oc == NDC - 1)
                    # 1/max(||h||, eps) per column
                    nm = rnp.tile([1, 2, 512], F32, tag="nm", name=f"nm{t}")
                    nc.scalar.activation(nm[:], ps_n[:], AF.Sqrt)
                    nm2 = rnp.tile([1, 2, 512], F32, tag="nm2", name=f"nm2{t}")
                    nc.vector.tensor_scalar(nm2[:], nm[:], EPS, None,
                                            op0=ALU.max)
                    rn_sb = rnp.tile([1, 2, 512], F32, tag="rn", name=f"rn{t}")
                    nc.vector.reciprocal(rn_sb[:], nm2[:])
                    # broadcast across partitions via rank-1 matmul
                    ps_rb = pp.tile([128, 2, 512], F32, tag="ps",
                                    name=f"psrb{t}")
                    for ihh in range(2):
                        nc.tensor.matmul(ps_rb[:, ihh, :], ones_row[:],
                                         rn_sb[:, ihh, :],
                                         start=True, stop=True)
                    for oc in range(NDC):
                        q, pair = divmod(oc, 2)
                        for ihh in range(2):
                            isl = slice(ihh * 512, ihh * 512 + 512)
                            nc.vector.tensor_tensor(
                                lns[t][oc][:, isl], hts[oc][:, ihh, :],
                                ps_rb[:, ihh, :], op=ALU.mult)
                        # x16-scaled fp8 copy in DoubleRow pair layout
                        nc.vector.tensor_scalar(
                            lf8[t][q][:, pair, :], lns[t][oc][:], SIGMA, None,
                            op0=ALU.mult)
                    for q in range(NQ):
                        nc.sync.dma_start(cc_in[t][q], lf8[t][q][:])
                    nc.gpsimd.collective_compute(
                        "AllGather", ALU.bypass,
                        replica_groups=[list(range(NCORES))],
                        ins=[cc_in[t].opt()], outs=[cc_out[t].opt()],
                    )

            # ---- diag12[i] = h1n_i . h2n_i (local, bf16 exact) --------
            diag_sb = accp.tile([1, NL], F32)
            for ih in range(NL // 512):
                isl = slice(ih * 512, ih * 512 + 512)
                ps_d = psn_pool.tile([1, 2, 512], F32, tag="pssm", name=f"psd{ih}")
                for dc in range(NDC):
                    pr = ep.tile([128, 2, 512], F32, tag="e", name=f"p12_{ih}{dc}")
                    nc.vector.tensor_tensor(pr[:, 0, :], lns[0][dc][:, isl],
                                            lns[1][dc][:, isl], op=ALU.mult)
                    nc.tensor.matmul(ps_d[:, 0, :], ones_col[:], pr[:, 0, :],
                                     start=dc == 0, stop=dc == NDC - 1)
                nc.vector.tensor_copy(diag_sb[:, isl], ps_d[:, 0, :])
            nc.sync.dma_start(out_diag[:, :], diag_sb[:])

            # ---- Phase C: row blocks of the three similarity matrices -
            # acc1[it]: cols 0..7 rowsum-parts of exp(2 S11), 8..15 of S12
            acc1 = [accp.tile([128, 16], F32, name=f"acc1_{it}")
                    for it in range(NIT)]
            acc22 = [accp.tile([128, 8], F32, name=f"acc22_{it}")
                     for it in range(NIT)]
            # partial column sums of exp(2 S12) over the 8 local row tiles,
            # still spread over 128 partitions; reduced after the loop
            csacc = accp.tile([128, NCORES, 2, 512], BF16)

            with tc.tile_pool(name="gpool", bufs=3) as gp:
                # pass 1: S11 (needs only the first AllGather)
                for rc in range(NCORES):
                    g = []
                    for q in range(NQ):
                        gt = gp.tile([128, NQ, NL], FP8, tag=f"g{q}",
                                     name=f"ga{rc}{q}")
                        nc.sync.dma_start(gt[:], cc_out[0][rc, q])
                        g.append(gt)
                    for it in range(NIT):
                        lsl = slice(it * 128, it * 128 + 128)
                        ps11 = pp.tile([128, 2, 512], F32, tag="ps",
                                       name=f"ps11_{rc}{it}")
                        for jhh in range(2):
                            for q in range(NQ):
                                nc.tensor.matmul(
                                    ps11[:, jhh, :], lf8[0][q][:, :, lsl],
                                    g[q][:, :, jhh * 512:(jhh + 1) * 512],
                                    start=q == 0, stop=q == NQ - 1,
                                    perf_mode=DR)
                        e11 = ep.tile([128, 2, 512], F32, tag="e12",
                                      name=f"e11_{rc}{it}")
                        nc.scalar.activation(e11[:], ps11[:], AF.Exp,
                                             scale=SCALE_DEV)
                        nc.vector.tensor_reduce(acc1[it][:, rc:rc + 1], e11[:],
                                                axis=mybir.AxisListType.XY,
                                                op=ALU.add)
                # pass 2: S22 + S12 (needs the second AllGather)
                for rc in range(NCORES):
                    g = []
                    for q in range(NQ):
                        gt = gp.tile([128, NQ, NL], FP8, tag=f"g{q}",
                                     name=f"gb{rc}{q}")
                        nc.sync.dma_start(gt[:], cc_out[1][rc, q])
                        g.append(gt)
                    for it in range(NIT):
                        lsl = slice(it * 128, it * 128 + 128)
                        ps22 = pp.tile([128, 2, 512], F32, tag="ps",
                                       name=f"ps22_{rc}{it}")
                        for jhh in range(2):
                            for q in range(NQ):
                                nc.tensor.matmul(
                                    ps22[:, jhh, :], lf8[1][q][:, :, lsl],
                                    g[q][:, :, jhh * 512:(jhh + 1) * 512],
                                    start=q == 0, stop=q == NQ - 1,
                                    perf_mode=DR)
                        e22 = ep.tile([128, 2, 512], BF16, tag="e",
                                      name=f"e22_{rc}{it}")
                        nc.scalar.activation(e22[:], ps22[:], AF.Exp,
                                             scale=SCALE_DEV,
                                             accum_out=acc22[it][:, rc:rc + 1])
                        ps12 = pp.tile([128, 2, 512], F32, tag="ps",
                                       name=f"ps12_{rc}{it}")
                        for jhh in range(2):
                            for q in range(NQ):
                                nc.tensor.matmul(
                                    ps12[:, jhh, :], lf8[0][q][:, :, lsl],
                                    g[q][:, :, jhh * 512:(jhh + 1) * 512],
                                    start=q == 0, stop=q == NQ - 1,
                                    perf_mode=DR)
                        e12 = ep.tile([128, 2, 512], BF16, tag="e12",
                                      name=f"e12_{rc}{it}")
                        nc.scalar.activation(e12[:], ps12[:], AF.Exp,
                                             scale=SCALE_DEV,
                                             accum_out=acc1[it][:, 8 + rc:9 + rc])
                        # column-sum partials accumulate on DVE
                        if it == 0:
                            nc.vector.tensor_copy(csacc[:, rc], e12[:])
                        else:
                            nc.vector.tensor_tensor(csacc[:, rc], csacc[:, rc],
                                                    e12[:], op=ALU.add)

            # partition-reduce the column-sum partials: [128, N] -> [1, N]
            for jb in range(N // 512):
                rc, jhh = divmod(jb, 2)
                ps_cs = psn_pool.tile([1, 2, 512], F32, tag="pssm",
                                      name=f"pscs{jb}")
                nc.tensor.matmul(ps_cs[:, 0, :], ones_col_bf[:],
                                 csacc[:, rc, jhh, :],
                                 start=True, stop=True)
                cs_st = accp.tile([1, 512], F32, tag="csst", bufs=2,
                                  name=f"csst{jb}")
                nc.vector.tensor_copy(cs_st[:], ps_cs[:, 0, :])
                nc.sync.dma_start(out_cs[:, jb * 512:(jb + 1) * 512], cs_st[:])

            # ---- final row-sum reduction ------------------------------
            rs1_sb = accp.tile([128, NIT], F32)
            rs22_sb = accp.tile([128, NIT], F32)
            for it in range(NIT):
                nc.vector.tensor_reduce(rs1_sb[:, it:it + 1], acc1[it][:],
                                        axis=mybir.AxisListType.X, op=ALU.add)
                nc.vector.tensor_reduce(rs22_sb[:, it:it + 1], acc22[it][:],
                                        axis=mybir.AxisListType.X, op=ALU.add)
            nc.sync.dma_start(out_rs1[:, :], rs1_sb[:])
            nc.sync.dma_start(out_rs22[:, :], rs22_sb[:])

    nc.compile()
    return nc


def _get_nc():
    if "nc" not in _CACHE:
        _CACHE["nc"] = _build()
    return _CACHE["nc"]


def kernel(z1, z2, index, fc1_w, fc1_b, fc2_w, fc2_b, **_unused):
    z1 = np.asarray(z1, np.float32)
    z2 = np.asarray(z2, np.float32)
    fc1_w = np.asarray(fc1_w, np.float32)
    fc1_b = np.asarray(fc1_b, np.float32)
    fc2_w = np.asarray(fc2_w, np.float32)
    fc2_b = np.asarray(fc2_b, np.float32)

    f8 = mybir.dt.np(FP8)

    def pack_dr(arr_t):  # [D, cols] -> [q, p, pair, cols] fp8
        d, cols = arr_t.shape
        a = arr_t.astype(f8).reshape(NQ, NQ, 128, cols).transpose(0, 2, 1, 3)
        return np.ascontiguousarray(a)

    z1t = np.ascontiguousarray(z1.T)  # [D, N]
    z2t = np.ascontiguousarray(z2.T)
    w1f8 = pack_dr(fc1_w * SIGMA)
    w2f8 = pack_dr(fc2_w * SIGMA)
    b1s = np.ascontiguousarray((SIGMA * fc1_b).reshape(D, 1))
    # fold the +1 shift of (elu+1) back out through layer 2
    b2s = np.ascontiguousarray(
        (SIGMA * (fc2_b - fc2_w.sum(axis=0))).reshape(D, 1))

    in_maps = []
    for r in range(NCORES):
        sl = slice(r * NL, (r + 1) * NL)
        in_maps.append({
            "z1f8": pack_dr(z1t[:, sl]),
            "z2f8": pack_dr(z2t[:, sl]),
            "w1f8": w1f8, "b1s": b1s, "w2f8": w2f8, "b2s": b2s,
        })

    nc = _get_nc()
    res = run_bass_kernel_spmd(nc, in_maps, list(range(NCORES)))

    E2 = np.exp(np.float64(1.0 / TAU))  # exp(2 * ||hn||^2), ||hn||^2 == 1
    cs_total = np.zeros(N, np.float64)
    for r in range(NCORES):
        cs_total += res.results[r]["out_cs"].reshape(N).astype(np.float64)

    total = 0.0
    for r in range(NCORES):
        out = res.results[r]
        # [128, NIT] with element [p, it] -> local row it*128 + p
        rs1 = out["out_rs1"].astype(np.float64).T.reshape(NL)
        rs22 = out["out_rs22"].astype(np.float64).T.reshape(NL)
        diag = out["out_diag"].astype(np.float64).reshape(NL)
        denom1 = rs1 - E2
        denom2 = rs22 - E2 + cs_total[r * NL:(r + 1) * NL]
        l_sum = 0.5 * (np.log(denom1) + np.log(denom2)) - (1.0 / TAU) * diag
        total += l_sum.sum()

    return np.float32(total / N)


# revision 25
# speedup vs baseline: 1.0237x; 1.0237x over previous
"""Distributed GRACE-style contrastive loss on 8 Trainium2 NeuronCores.

Math (reference):
    h = elu(z @ W1 + b1) @ W2 + b2           for z1, z2    -> h1, h2
    hn = h / max(||h||_row, eps)
    S11 = h1n @ h1n.T, S22 = h2n @ h2n.T, S12 = h1n @ h2n.T   (N x N)
    denom1_i = sum_j e^{2 S11_ij} + sum_j e^{2 S12_ij} - e^{2 S11_ii}
    denom2_i = sum_j e^{2 S22_ij} + sum_j e^{2 S12_ji} - e^{2 S22_ii}
    loss = mean_i [ 0.5 (log denom1_i + log denom2_i) - 2 S12_ii ]

Strategy: shard rows across 8 cores.  Each core projects+normalizes its
1024 rows (transposed layout, features on partitions; fp8 DoubleRow
matmuls with x16-scaled weights, biases and the 1/16 descale folded
into the inter-layer vector ops), emits the normalized block both as
bf16 (for the exact S12 diagonal) and as x16-scaled fp8e4 in DoubleRow
pair layout, AllGathers the fp8 blocks (one collective per tensor so
the first overlaps the second projection), then computes its row-block
of the three similarity matrices with fp8 DoubleRow matmuls (K=256 per
instruction) and fused exp+row-reduction on the scalar engine.  The
x256 dot-product scale is folded into the exp.  The S11 pass only needs
the first AllGather, so it runs while the second is in flight.  Column
sums of exp(2 S12) accumulate on the vector engine and are
partition-reduced at the end; the final log/mean combine runs on the
host from tiny per-core outputs.
"""

import sys

sys.path.insert(0, "/opt/trn_rl_repo")

import numpy as np
from concourse import bacc, mybir, tile
from concourse.bass_utils import run_bass_kernel_spmd

F32 = mybir.dt.float32
BF16 = mybir.dt.bfloat16
FP8 = mybir.dt.float8e4
AF = mybir.ActivationFunctionType
ALU = mybir.AluOpType
DR = mybir.MatmulPerfMode.DoubleRow

N = 8192          # total rows
D = 512           # hidden dim (= proj dim)
NCORES = 8
NL = N // NCORES  # 1024 local rows per core
TAU = 0.5
SIGMA = 16.0      # fp8 pre-scale; S accumulates SIGMA^2 * S_true
SCALE_DEV = (1.0 / TAU) / (SIGMA * SIGMA)  # exp scale on device
NDC = D // 128    # 4 feature chunks of 128 partitions
NQ = 2            # two K=256 DoubleRow groups
NIT = NL // 128   # 8 local row tiles of 128
EPS = 1e-12

_CACHE = {}


def _build():
    nc = bacc.Bacc("TRN2", target_bir_lowering=False, debug=False,
                   num_devices=NCORES)

    # ---- I/O ----------------------------------------------------------
    # z and the x16-scaled weights come pre-packed in fp8 DoubleRow pair
    # layout [q, p, pair, cols] with contraction index d = q*256 + pair*128 + p
    z1t_d = nc.declare_dram_parameter("z1f8", [NQ, 128, NQ, NL], FP8, isOutput=False)
    z2t_d = nc.declare_dram_parameter("z2f8", [NQ, 128, NQ, NL], FP8, isOutput=False)
    w1_d = nc.declare_dram_parameter("w1f8", [NQ, 128, NQ, D], FP8, isOutput=False)
    # biases pre-multiplied by 16 on the host; (psum + 16 b)/16 descales the
    # x16 fp8 weight scale in a single two-scalar vector op
    b1_d = nc.declare_dram_parameter("b1s", [D, 1], F32, isOutput=False)
    w2_d = nc.declare_dram_parameter("w2f8", [NQ, 128, NQ, D], FP8, isOutput=False)
    # b2 adjusted on host: b2 - colsum(W2) (folds the elu()+1 shift back out)
    b2_d = nc.declare_dram_parameter("b2s", [D, 1], F32, isOutput=False)

    # rowsum(exp 2*S11)+rowsum(exp 2*S12) and rowsum(exp 2*S22); [p, it]
    out_rs1 = nc.declare_dram_parameter("out_rs1", [128, NIT], F32, isOutput=True)
    out_rs22 = nc.declare_dram_parameter("out_rs22", [128, NIT], F32, isOutput=True)
    out_diag = nc.declare_dram_parameter("out_diag", [1, NL], F32, isOutput=True)
    out_cs = nc.declare_dram_parameter("out_cs", [1, N], F32, isOutput=True)

    with tile.TileContext(nc) as tc:
        with (
            tc.tile_pool(name="const", bufs=1) as constp,
            tc.tile_pool(name="locals", bufs=1) as localp,
            tc.tile_pool(name="accs", bufs=1) as accp,
            tc.tile_pool(name="escratch", bufs=8) as ep,
            tc.tile_pool(name="psbig", bufs=3, space="PSUM") as pp,
            tc.tile_pool(name="pssmall", bufs=1, space="PSUM") as psn_pool,
            tc.tile_pool(name="dram", bufs=1, space="DRAM") as dramp,
        ):
            ones_col = constp.tile([128, 1], F32)
            nc.vector.memset(ones_col[:], 1.0)
            ones_row = constp.tile([1, 128], F32)
            nc.vector.memset(ones_row[:], 1.0)
            ones_col_bf = constp.tile([128, 1], BF16)
            nc.vector.memset(ones_col_bf[:], 1.0)

            w1_sb = []
            w2_sb = []
            for q in range(NQ):
                w1t = constp.tile([128, NQ, D], FP8, name=f"w1_{q}")
                nc.sync.dma_start(w1t[:], w1_d[q])
                w1_sb.append(w1t)
                w2t = constp.tile([128, NQ, D], FP8, name=f"w2_{q}")
                nc.sync.dma_start(w2t[:], w2_d[q])
                w2_sb.append(w2t)
            b1_sb = constp.tile([128, NDC], F32)
            b2_sb = constp.tile([128, NDC], F32)
            for dc in range(NDC):
                nc.gpsimd.dma_start(b1_sb[:, dc:dc + 1], b1_d[dc * 128:(dc + 1) * 128, :])
                nc.gpsimd.dma_start(b2_sb[:, dc:dc + 1], b2_d[dc * 128:(dc + 1) * 128, :])

            # normalized local embeddings, transposed: bf16 [d, i] for the
            # diagonal, and x16-scaled fp8 in DoubleRow pair layout
            # [p, pair, i] with d = q*256 + pair*128 + p
            lns = [[localp.tile([128, NL], BF16, name=f"ln{t}_{dc}")
                    for dc in range(NDC)] for t in range(2)]
            lf8 = [[localp.tile([128, NQ, NL], FP8, name=f"lf8_{t}_{q}")
                    for q in range(NQ)] for t in range(2)]

            cc_in = [dramp.tile([NQ, 128, NQ, NL], FP8, name=f"cc_in{t}")
                     for t in range(2)]
            cc_out = [dramp.tile([NCORES, NQ, 128, NQ, NL], FP8,
                                 addr_space="Shared", name=f"cc_out{t}")
                      for t in range(2)]

            # ---- Phase A: projection + normalize ----------------------
            # ACT kept to Exp/Sqrt only (table swaps are expensive);
            # relu/square run on the vector engine.
            with (
                tc.tile_pool(name="zpool", bufs=2) as zp,
                tc.tile_pool(name="elupool", bufs=2) as elup,
                tc.tile_pool(name="hpool", bufs=2) as hp,
                tc.tile_pool(name="rnpool", bufs=2) as rnp,
            ):
                for t, zt_d in ((0, z1t_d), (1, z2t_d)):
                    zt = []
                    for q in range(NQ):
                        z = zp.tile([128, NQ, NL], FP8, tag=f"z{q}", name=f"z{t}_{q}")
                        nc.sync.dma_start(z[:], zt_d[q])
                        zt.append(z)
                    # layer 1: a.T[p, i] = 16 W1[d,p].T @ z.T[d, i], fp8 DR
                    elus = [elup.tile([128, NQ, NL], FP8, tag=f"el{q}",
                                      name=f"el{t}{q}") for q in range(NQ)]
                    for pc in range(NDC):
                        ps_a = pp.tile([128, 2, 512], F32, tag="ps",
                                       name=f"psa{t}{pc}")
                        for ihh in range(2):
                            for q in range(NQ):
                                nc.tensor.matmul(
                                    ps_a[:, ihh, :],
                                    w1_sb[q][:, :, pc * 128:(pc + 1) * 128],
                                    zt[q][:, :, ihh * 512:(ihh + 1) * 512],
                                    start=q == 0, stop=q == NQ - 1,
                                    perf_mode=DR)
                        # elu(x)+1 = relu(x) + exp(min(x, 0)),  x = (a+16b1)/16
                        xp_sb = ep.tile([128, 2, 512], F32, tag="e",
                                        name=f"xp{t}{pc}")
                        nc.vector.tensor_scalar(
                            xp_sb[:], ps_a[:], b1_sb[:, pc:pc + 1], 1.0 / SIGMA,
                            op0=ALU.add, op1=ALU.mult)
                        r_sb = ep.tile([128, 2, 512], F32, tag="e",
                                       name=f"r{t}{pc}")
                        nc.vector.tensor_scalar(
                            r_sb[:], xp_sb[:], 0.0, None, op0=ALU.max)
                        m_sb = ep.tile([128, 2, 512], F32, tag="e",
                                       name=f"m{t}{pc}")
                        nc.vector.tensor_scalar(
                            m_sb[:], xp_sb[:], 0.0, None, op0=ALU.min)
                        x_sb = ep.tile([128, 2, 512], F32, tag="e",
                                       name=f"x{t}{pc}")
                        nc.scalar.activation(x_sb[:], m_sb[:], AF.Exp)
                        q, pair = divmod(pc, 2)
                        for ihh in range(2):
                            nc.vector.tensor_tensor(
                                elus[q][:, pair, ihh * 512:(ihh + 1) * 512],
                                r_sb[:, ihh, :], x_sb[:, ihh, :], op=ALU.add)
                    # layer 2: h.T[o, i] = 16 W2[p,o].T @ elup1.T[p, i], fp8 DR
                    ps_n = psn_pool.tile([1, 2, 512], F32, tag="pssm",
                                         name=f"psn{t}")
                    hts = []
                    for oc in range(NDC):
                        ps_h = pp.tile([128, 2, 512], F32, tag="ps",
                                       name=f"psh{t}{oc}")
                        for ihh in range(2):
                            for q in range(NQ):
                                nc.tensor.matmul(
                                    ps_h[:, ihh, :],
                                    w2_sb[q][:, :, oc * 128:(oc + 1) * 128],
                                    elus[q][:, :, ihh * 512:(ihh + 1) * 512],
                                    start=q == 0, stop=q == NQ - 1,
                                    perf_mode=DR)
                        ht = hp.tile([128, 2, 512], F32, tag=f"h{oc}",
                                     name=f"h{t}{oc}")
                        nc.vector.tensor_scalar(
                            ht[:], ps_h[:], b2_sb[:, oc:oc + 1], 1.0 / SIGMA,
                            op0=ALU.add, op1=ALU.mult)
                        sq = ep.tile([128, 2, 512], F32, tag="e",
                                     name=f"sq{t}{oc}")
                        nc.vector.tensor_tensor(sq[:], ht[:], ht[:],
                                                op=ALU.mult)
                        hts.append(ht)
                        for ihh in range(2):
                            nc.tensor.matmul(ps_n[:, ihh, :], ones_col[:],
                                             sq[:, ihh, :],
                                             start=oc == 0, stop=oc == NDC - 1)
                    # 1/max(||h||, eps) per column
                    nm = rnp.tile([1, 2, 512], F32, tag="nm", name=f"nm{t}")
                    nc.scalar.activation(nm[:], ps_n[:], AF.Sqrt)
                    nm2 = rnp.tile([1, 2, 512], F32, tag="nm2", name=f"nm2{t}")
                    nc.vector.tensor_scalar(nm2[:], nm[:], EPS, None,
                                            op0=ALU.max)
                    rn_sb = rnp.tile([1, 2, 512], F32, tag="rn", name=f"rn{t}")
                    nc.vector.reciprocal(rn_sb[:], nm2[:])
                    # broadcast across partitions via rank-1 matmul
                    ps_rb = pp.tile([128, 2, 512], F32, tag="ps",
                                    name=f"psrb{t}")
                    for ihh in range(2):
                        nc.tensor.matmul(ps_rb[:, ihh, :], ones_row[:],
                                         rn_sb[:, ihh, :],
                                         start=True, stop=True)
                    for oc in range(NDC):
                        q, pair = divmod(oc, 2)
                        for ihh in range(2):
                            isl = slice(ihh * 512, ihh * 512 + 512)
                            nc.vector.tensor_tensor(
                                lns[t][oc][:, isl], hts[oc][:, ihh, :],
                                ps_rb[:, ihh, :], op=ALU.mult)
                        # x16-scaled fp8 copy in DoubleRow pair layout
                        nc.vector.tensor_scalar(
                            lf8[t][q][:, pair, :], lns[t][oc][:], SIGMA, None,
                            op0=ALU.mult)
                    for q in range(NQ):
                        nc.sync.dma_start(cc_in[t][q], lf8[t][q][:])
                    nc.gpsimd.collective_compute(
                        "AllGather", ALU.bypass,
                        replica_groups=[list(range(NCORES))],
                        ins=[cc_in[t].opt()], outs=[cc_out[t].opt()],
                    )

            # ---- diag12[i] = h1n_i . h2n_i (local, bf16 exact) --------
            diag_sb = accp.tile([1, NL], F32)
            for ih in range(NL // 512):
                isl = slice(ih * 512, ih * 512 + 512)
                ps_d = psn_pool.tile([1, 2, 512], F32, tag="pssm", name=f"psd{ih}")
                for dc in range(NDC):
                    pr = ep.tile([128, 2, 512], F32, tag="e", name=f"p12_{ih}{dc}")
                    nc.vector.tensor_tensor(pr[:, 0, :], lns[0][dc][:, isl],
                                            lns[1][dc][:, isl], op=ALU.mult)
                    nc.tensor.matmul(ps_d[:, 0, :], ones_col[:], pr[:, 0, :],
                                     start=dc == 0, stop=dc == NDC - 1)
                nc.vector.tensor_copy(diag_sb[:, isl], ps_d[:, 0, :])
            nc.sync.dma_start(out_diag[:, :], diag_sb[:])

            # ---- Phase C: row blocks of the three similarity matrices -
            # acc1[it]: cols 0..7 rowsum-parts of exp(2 S11), 8..15 of S12
            acc1 = [accp.tile([128, 16], F32, name=f"acc1_{it}")
                    for it in range(NIT)]
            acc22 = [accp.tile([128, 8], F32, name=f"acc22_{it}")
                     for it in range(NIT)]
            # partial column sums of exp(2 S12) over the 8 local row tiles,
            # still spread over 128 partitions; reduced after the loop
            csacc = accp.tile([128, NCORES, 2, 512], BF16)

            with tc.tile_pool(name="gpool", bufs=3) as gp:
                # pass 1: S11 (needs only the first AllGather)
                for rc in range(NCORES):
                    g = []
                    for q in range(NQ):
                        gt = gp.tile([128, NQ, NL], FP8, tag=f"g{q}",
                                     name=f"ga{rc}{q}")
                        nc.sync.dma_start(gt[:], cc_out[0][rc, q])
                        g.append(gt)
                    for it in range(NIT):
                        lsl = slice(it * 128, it * 128 + 128)
                        ps11 = pp.tile([128, 2, 512], F32, tag="ps",
                                       name=f"ps11_{rc}{it}")
                        for jhh in range(2):
                            for q in range(NQ):
                                nc.tensor.matmul(
                                    ps11[:, jhh, :], lf8[0][q][:, :, lsl],
                                    g[q][:, :, jhh * 512:(jhh + 1) * 512],
                                    start=q == 0, stop=q == NQ - 1,
                                    perf_mode=DR)
                        e11 = ep.tile([128, 2, 512], F32, tag="e12",
                                      name=f"e11_{rc}{it}")
                        nc.scalar.activation(e11[:], ps11[:], AF.Exp,
                                             scale=SCALE_DEV)
                        nc.vector.tensor_reduce(acc1[it][:, rc:rc + 1], e11[:],
                                                axis=mybir.AxisListType.XY,
                                                op=ALU.add)
                # pass 2: S22 + S12 (needs the second AllGather)
                for rc in range(NCORES):
                    g = []
                    for q in range(NQ):
                        gt = gp.tile([128, NQ, NL], FP8, tag=f"g{q}",
                                     name=f"gb{rc}{q}")
                        nc.sync.dma_start(gt[:], cc_out[1][rc, q])
                        g.append(gt)
                    for it in range(NIT):
                        lsl = slice(it * 128, it * 128 + 128)
                        ps22 = pp.tile([128, 2, 512], F32, tag="ps",
                                       name=f"ps22_{rc}{it}")
                        for jhh in range(2):
                            for q in range(NQ):
                                nc.tensor.matmul(
                                    ps22[:, jhh, :], lf8[1][q][:, :, lsl],
                                    g[q][:, :, jhh * 512:(jhh + 1) * 512],
                                    start=q == 0, stop=q == NQ - 1,
                                    perf_mode=DR)
                        e22 = ep.tile([128, 2, 512], F32, tag="e",
                                      name=f"e22_{rc}{it}")
                        nc.scalar.activation(e22[:], ps22[:], AF.Exp,
                                             scale=SCALE_DEV,
                                             accum_out=acc22[it][:, rc:rc + 1])
                        ps12 = pp.tile([128, 2, 512], F32, tag="ps",
                                       name=f"ps12_{rc}{it}")
                        for jhh in range(2):
                            for q in range(NQ):
                                nc.tensor.matmul(
                                    ps12[:, jhh, :], lf8[0][q][:, :, lsl],
                                    g[q][:, :, jhh * 512:(jhh + 1) * 512],
                                    start=q == 0, stop=q == NQ - 1,
                                    perf_mode=DR)
                        e12 = ep.tile([128, 2, 512], F32, tag="e12",
                                      name=f"e12_{rc}{it}")
                        nc.scalar.activation(e12[:], ps12[:], AF.Exp,
                                             scale=SCALE_DEV,
                                             accum_out=acc1[it][:, 8 + rc:9 + rc])
                        # column-sum partials accumulate on DVE
                        if it == 0:
                            nc.vector.tensor_copy(csacc[:, rc], e12[:])
                        else:
                            nc.vector.tensor_tensor(csacc[:, rc], csacc[:, rc],
                                                    e12[:], op=ALU.add)

            # partition-reduce the column-sum partials: [128, N] -> [1, N]
            for jb in range(N // 512):
                rc, jhh = divmod(jb, 2)
                ps_cs = psn_pool.tile([1, 2, 512], F32, tag="pssm",
                                      name=f"pscs{jb}")
                nc.tensor.matmul(ps_cs[:, 0, :], ones_col_bf[:],
                                 csacc[:, rc, jhh, :],
                                 start=True, stop=True)
                cs_st = accp.tile([1, 512], F32, tag="csst", bufs=2,
                                  name=f"csst{jb}")
                nc.vector.tensor_copy(cs_st[:], ps_cs[:, 0, :])
                nc.sync.dma_start(out_cs[:, jb * 512:(jb + 1) * 512], cs_st[:])

            # ---- final row-sum reduction ------------------------------
            rs1_sb = accp.tile([128, NIT], F32)
            rs22_sb = accp.tile([128, NIT], F32)
            for it in range(NIT):
                nc.vector.tensor_reduce(rs1_sb[:, it:it + 1], acc1[it][:],
                                        axis=mybir.AxisListType.X, op=ALU.add)
                nc.vector.tensor_reduce(rs22_sb[:, it:it + 1], acc22[it][:],
                                        axis=mybir.AxisListType.X, op=ALU.add)
            nc.sync.dma_start(out_rs1[:, :], rs1_sb[:])
            nc.sync.dma_start(out_rs22[:, :], rs22_sb[:])

    nc.compile()
    return nc


def _get_nc():
    if "nc" not in _CACHE:
        _CACHE["nc"] = _build()
    return _CACHE["nc"]


def kernel(z1, z2, index, fc1_w, fc1_b, fc2_w, fc2_b, **_unused):
    z1 = np.asarray(z1, np.float32)
    z2 = np.asarray(z2, np.float32)
    fc1_w = np.asarray(fc1_w, np.float32)
    fc1_b = np.asarray(fc1_b, np.float32)
    fc2_w = np.asarray(fc2_w, np.float32)
    fc2_b = np.asarray(fc2_b, np.float32)

    f8 = mybir.dt.np(FP8)

    def pack_dr(arr_t):  # [D, cols] -> [q, p, pair, cols] fp8
        d, cols = arr_t.shape
        a = arr_t.astype(f8).reshape(NQ, NQ, 128, cols).transpose(0, 2, 1, 3)
        return np.ascontiguousarray(a)

    z1t = np.ascontiguousarray(z1.T)  # [D, N]
    z2t = np.ascontiguousarray(z2.T)
    w1f8 = pack_dr(fc1_w * SIGMA)
    w2f8 = pack_dr(fc2_w * SIGMA)
    b1s = np.ascontiguousarray((SIGMA * fc1_b).reshape(D, 1))
    # fold the +1 shift of (elu+1) back out through layer 2
    b2s = np.ascontiguousarray(
        (SIGMA * (fc2_b - fc2_w.sum(axis=0))).reshape(D, 1))

    in_maps = []
    for r in range(NCORES):
        sl = slice(r * NL, (r + 1) * NL)
        in_maps.append({
            "z1f8": pack_dr(z1t[:, sl]),
            "z2f8": pack_dr(z2t[:, sl]),
            "w1f8": w1f8, "b1s": b1s, "w2f8": w2f8, "b2s": b2s,
        })

    nc = _get_nc()
    res = run_bass_kernel_spmd(nc, in_maps, list(range(NCORES)))

    E2 = np.exp(np.float64(1.0 / TAU))  # exp(2 * ||hn||^2), ||hn||^2 == 1
    cs_total = np.zeros(N, np.float64)
    for r in range(NCORES):
        cs_total += res.results[r]["out_cs"].reshape(N).astype(np.float64)

    total = 0.0
    for r in range(NCORES):
        out = res.results[r]
        # [128, NIT] with element [p, it] -> local row it*128 + p
        rs1 = out["out_rs1"].astype(np.float64).T.reshape(NL)
        rs22 = out["out_rs22"].astype(np.float64).T.reshape(NL)
        diag = out["out_diag"].astype(np.float64).reshape(NL)
        denom1 = rs1 - E2
        denom2 = rs22 - E2 + cs_total[r * NL:(r + 1) * NL]
        l_sum = 0.5 * (np.log(denom1) + np.log(denom2)) - (1.0 / TAU) * diag
        total += l_sum.sum()

    return np.float32(total / N)
